# revision 15
# baseline (speedup 1.0000x reference)
"""Trainium2 Bass kernel for nn_Encoder (dense_mlp).

The reference network returns only `flights_out`, which depends solely on
state[:, :1024] (64 flights x 16 features per row).  The whole flight path
(4 input encoders -> f1,f2,f3 -> df1,df2,df3 -> 4 output decoders) contains
no nonlinearity, so it collapses to a single affine map per flight vector:

    y[b, f, :] = x[b, f, :] @ Wtotal + btotal          (Wtotal: 16x16)

We evaluate it as a 128-wide matmul by block-diagonalizing 8 flights:
W8 = blockdiag(Wtotal x 8) [128, 128].  Each core handles 2048 batch rows:

    yT[128c:128c+128, rows] = W8.T @ xT[128c:128c+128, rows] + bias

Host passes xT (feature-major transposed slice) so the device only runs
full-rate streaming matmuls with W8 stationary; outputs come back
feature-major and are transposed on host.
"""

import numpy as np

B = 16384
N_CORES = 8
ROWS = B // N_CORES          # 2048 rows per core
FLIGHT_COLS = 1024           # 64 flights * 16 features
NCHUNK = FLIGHT_COLS // 128  # 8 feature chunks of 128
MM_N = 512                   # fp32 moving-operand max per matmul
NSPLIT = ROWS // MM_N        # 4 matmuls per chunk

_NC_CACHE = {}


def _collapse_params(params):
    """Fold the whole affine chain into (Wtotal [16,16], btotal [16]) in f64."""
    def g(name):
        W, b = params[name]
        return np.asarray(W, np.float64), np.asarray(b, np.float64)

    Wenc = np.zeros((16, 16)); benc = np.zeros(16)
    Wa, ba = g('airConv'); Wf, bf = g('flightTypeConv')
    Wt, bt = g('timeConv'); We, be = g('etaConv')
    Wenc[0:4, 0:4] = Wa;   benc[0:4] = ba
    Wenc[4:14, 4:8] = Wf;  benc[4:8] = bf
    Wenc[14:15, 8:12] = Wt; benc[8:12] = bt
    Wenc[15:16, 12:16] = We; benc[12:16] = be

    W, b = Wenc, benc
    for name in ('f1', 'f2', 'f3', 'df1', 'df2', 'df3'):
        Wn, bn = g(name)
        W = W @ Wn
        b = b @ Wn + bn

    Wdec = np.zeros((16, 16)); bdec = np.zeros(16)
    Wa, ba = g('airDe'); Wf, bf = g('ftDe'); Wt, bt = g('timeDe'); We, be = g('etaDe')
    Wdec[0:4, 0:4] = Wa;    bdec[0:4] = ba
    Wdec[4:8, 4:14] = Wf;   bdec[4:14] = bf
    Wdec[8:12, 14:15] = Wt; bdec[14:15] = bt
    Wdec[12:16, 15:16] = We; bdec[15:16] = be

    W = W @ Wdec
    b = b @ Wdec + bdec
    return W, b


def _build_nc(xin_bufs=4, yout_bufs=4, mm_dtype='float32', copy_split=False, out_engine='scalar'):
    import concourse.bacc as bacc
    import concourse.tile as tile
    import concourse.mybir as mybir

    nc = bacc.Bacc(trn_type="TRN2")
    f32 = mybir.dt.float32
    # float32r is bit-identical fp32 in memory but streams through the PE at
    # 1 cycle/row (vs 4 for float32) when the moving dim is >= 256
    mdt = getattr(mybir.dt, mm_dtype)
    xt = nc.dram_tensor("xt", [FLIGHT_COLS, ROWS], mdt, kind="ExternalInput")
    w8 = nc.dram_tensor("w8", [128, 128], mdt, kind="ExternalInput")
    bias = nc.dram_tensor("bias", [128, 1], f32, kind="ExternalInput")
    yt = nc.dram_tensor("yt", [FLIGHT_COLS, ROWS], f32, kind="ExternalOutput")

    with tile.TileContext(nc) as tc:
        with tc.tile_pool(name="const", bufs=1) as cpool, \
             tc.tile_pool(name="xin", bufs=xin_bufs) as xpool, \
             tc.tile_pool(name="yout", bufs=yout_bufs) as ypool, \
             tc.tile_pool(name="ps", bufs=8, space="PSUM") as pspool:
            wt = cpool.tile([128, 128], mdt)
            nc.sync.dma_start(wt[:], w8[:, :])
            bt = cpool.tile([128, 1], f32)
            nc.sync.dma_start(bt[:], bias[:, :])
            for c in range(NCHUNK):
                # each [128, ROWS] chunk of xt/yt is one fully-contiguous
                # 1 MiB DRAM block -> peak DMA efficiency
                xtile = xpool.tile([128, ROWS], mdt)
                nc.sync.dma_start(xtile[:], xt[c * 128:(c + 1) * 128, :])
                ytile = ypool.tile([128, ROWS], f32)
                for n in range(NSPLIT):
                    ps = pspool.tile([128, MM_N], f32)
                    nc.tensor.matmul(
                        ps[:], wt[:], xtile[:, n * MM_N:(n + 1) * MM_N],
                        start=True, stop=True,
                    )
                    dst = ytile[:, n * MM_N:(n + 1) * MM_N]
                    # split the PSUM->SBUF bias-copies across DVE and ACT so
                    # neither engine paces out-tile production
                    if (not copy_split) or n % 2 == 0:
                        nc.vector.tensor_scalar_add(dst, ps[:], bt[:])
                    else:
                        nc.scalar.activation(
                            dst, ps[:],
                            mybir.ActivationFunctionType.Identity, bias=bt[:])
                # out-DMAs issue via GpSimd's SWDGE queue: keeps their
                # dependency waits off the Sync (in-DMA) and Scalar (copy)
                # queues entirely
                out_eng = getattr(nc, out_engine)
                out_eng.dma_start(yt[c * 128:(c + 1) * 128, :], ytile[:])
    nc.compile()
    return nc


def _get_nc():
    if 'nc' not in _NC_CACHE:
        _NC_CACHE['nc'] = _build_nc()
    return _NC_CACHE['nc']


def _run(in_maps, **kwargs):
    from concourse.bass_utils import run_bass_kernel_spmd
    return run_bass_kernel_spmd(_get_nc(), in_maps, core_ids=list(range(N_CORES)),
                                **kwargs)


def _make_in_maps(state, params):
    Wtot, btot = _collapse_params(params)
    W8 = np.zeros((128, 128), np.float64)
    for f in range(8):
        W8[f * 16:(f + 1) * 16, f * 16:(f + 1) * 16] = Wtot
    W8 = W8.astype(np.float32)
    bias128 = np.tile(btot, 8).astype(np.float32).reshape(128, 1)

    state = np.asarray(state, np.float32)
    in_maps = []
    for i in range(N_CORES):
        xs = state[i * ROWS:(i + 1) * ROWS, :FLIGHT_COLS]
        in_maps.append({
            'xt': np.ascontiguousarray(xs.T),
            'w8': W8,
            'bias': bias128,
        })
    return in_maps


def _assemble(results):
    out = np.empty((B, FLIGHT_COLS), np.float32)
    for i in range(N_CORES):
        out[i * ROWS:(i + 1) * ROWS] = results[i]['yt'].T
    return out


def kernel(state, params):
    in_maps = _make_in_maps(state, params)
    last_err = None
    for attempt in range(3):
        try:
            res = _run(in_maps)
            return _assemble(res.results)
        except Exception as e:  # transient NRT device errors recover on retry
            last_err = e
            _NC_CACHE.clear()
            import time
            time.sleep(2.0 * (attempt + 1))
    raise last_err


# revision 19
# speedup vs baseline: 1.0645x; 1.0645x over previous
"""Trainium2 Bass kernel for nn_Encoder (dense_mlp).

The reference network returns only `flights_out`, which depends solely on
state[:, :1024] (64 flights x 16 features per row).  The whole flight path
(4 input encoders -> f1,f2,f3 -> df1,df2,df3 -> 4 output decoders) contains
no nonlinearity, so it collapses to a single affine map per flight vector:

    y[b, f, :] = x[b, f, :] @ Wtotal + btotal          (Wtotal: 16x16)

We evaluate it as a 128-wide matmul by block-diagonalizing 8 flights:
W8 = blockdiag(Wtotal x 8) [128, 128].  Each core handles 2048 batch rows:

    yT[128c:128c+128, rows] = W8.T @ xT[128c:128c+128, rows] + bias

Host passes xT (feature-major transposed slice) so the device only runs
full-rate streaming matmuls with W8 stationary; outputs come back
feature-major and are transposed on host.
"""

import numpy as np

B = 16384
N_CORES = 8
ROWS = B // N_CORES          # 2048 rows per core
FLIGHT_COLS = 1024           # 64 flights * 16 features
NCHUNK = FLIGHT_COLS // 128  # 8 feature chunks of 128
MM_N = 512                   # fp32 moving-operand max per matmul
NSPLIT = ROWS // MM_N        # 4 matmuls per chunk

_NC_CACHE = {}


def _collapse_params(params):
    """Fold the whole affine chain into (Wtotal [16,16], btotal [16]) in f64."""
    def g(name):
        W, b = params[name]
        return np.asarray(W, np.float64), np.asarray(b, np.float64)

    Wenc = np.zeros((16, 16)); benc = np.zeros(16)
    Wa, ba = g('airConv'); Wf, bf = g('flightTypeConv')
    Wt, bt = g('timeConv'); We, be = g('etaConv')
    Wenc[0:4, 0:4] = Wa;   benc[0:4] = ba
    Wenc[4:14, 4:8] = Wf;  benc[4:8] = bf
    Wenc[14:15, 8:12] = Wt; benc[8:12] = bt
    Wenc[15:16, 12:16] = We; benc[12:16] = be

    W, b = Wenc, benc
    for name in ('f1', 'f2', 'f3', 'df1', 'df2', 'df3'):
        Wn, bn = g(name)
        W = W @ Wn
        b = b @ Wn + bn

    Wdec = np.zeros((16, 16)); bdec = np.zeros(16)
    Wa, ba = g('airDe'); Wf, bf = g('ftDe'); Wt, bt = g('timeDe'); We, be = g('etaDe')
    Wdec[0:4, 0:4] = Wa;    bdec[0:4] = ba
    Wdec[4:8, 4:14] = Wf;   bdec[4:14] = bf
    Wdec[8:12, 14:15] = Wt; bdec[14:15] = bt
    Wdec[12:16, 15:16] = We; bdec[15:16] = be

    W = W @ Wdec
    b = b @ Wdec + bdec
    return W, b


def _build_nc(xin_bufs=4, yout_bufs=4, mm_dtype='float32', copy_split=False, out_engine='scalar',
              in_split_chunks=0, out_split=False):
    import concourse.bacc as bacc
    import concourse.tile as tile
    import concourse.mybir as mybir

    nc = bacc.Bacc(trn_type="TRN2")
    f32 = mybir.dt.float32
    # float32r is bit-identical fp32 in memory but streams through the PE at
    # 1 cycle/row (vs 4 for float32) when the moving dim is >= 256
    mdt = getattr(mybir.dt, mm_dtype)
    xt = nc.dram_tensor("xt", [FLIGHT_COLS, ROWS], mdt, kind="ExternalInput")
    w8 = nc.dram_tensor("w8", [128, 128], mdt, kind="ExternalInput")
    bias = nc.dram_tensor("bias", [128, 1], f32, kind="ExternalInput")
    yt = nc.dram_tensor("yt", [FLIGHT_COLS, ROWS], f32, kind="ExternalOutput")

    with tile.TileContext(nc) as tc:
        with tc.tile_pool(name="const", bufs=1) as cpool, \
             tc.tile_pool(name="xin", bufs=xin_bufs) as xpool, \
             tc.tile_pool(name="yout", bufs=yout_bufs) as ypool, \
             tc.tile_pool(name="ps", bufs=8, space="PSUM") as pspool:
            wt = cpool.tile([128, 128], mdt)
            nc.sync.dma_start(wt[:], w8[:, :])
            bt = cpool.tile([128, 1], f32)
            nc.sync.dma_start(bt[:], bias[:, :])
            for c in range(NCHUNK):
                # each [128, ROWS] chunk of xt/yt is one fully-contiguous
                # 1 MiB DRAM block -> peak DMA efficiency
                xtile = xpool.tile([128, ROWS], mdt)
                if c < in_split_chunks:
                    # quarter the leading chunks across 4 DMA queues so they
                    # land in chunk order at full aggregate rate (a single
                    # 1 MiB transfer rides one queue at ~100-130 GB/s and
                    # delays the first matmul by ~4 us)
                    for q in range(4):
                        nc.sync.dma_start(
                            xtile[:, q * MM_N:(q + 1) * MM_N],
                            xt[c * 128:(c + 1) * 128, q * MM_N:(q + 1) * MM_N])
                else:
                    nc.sync.dma_start(xtile[:], xt[c * 128:(c + 1) * 128, :])
                ytile = ypool.tile([128, ROWS], f32)
                for n in range(NSPLIT):
                    ps = pspool.tile([128, MM_N], f32)
                    nc.tensor.matmul(
                        ps[:], wt[:], xtile[:, n * MM_N:(n + 1) * MM_N],
                        start=True, stop=True,
                    )
                    dst = ytile[:, n * MM_N:(n + 1) * MM_N]
                    # split the PSUM->SBUF bias-copies across DVE and ACT so
                    # neither engine paces out-tile production
                    if (not copy_split) or n % 2 == 0:
                        nc.vector.tensor_scalar_add(dst, ps[:], bt[:])
                    else:
                        nc.scalar.activation(
                            dst, ps[:],
                            mybir.ActivationFunctionType.Identity, bias=bt[:])
                # out-DMAs issue via GpSimd's SWDGE queue: keeps their
                # dependency waits off the Sync (in-DMA) and Scalar (copy)
                # queues entirely
                out_eng = getattr(nc, out_engine)
                if out_split:
                    # quartered outs spread across queues: smoother out
                    # stream and a 4x-parallel final drain
                    for q in range(4):
                        out_eng.dma_start(
                            yt[c * 128:(c + 1) * 128, q * MM_N:(q + 1) * MM_N],
                            ytile[:, q * MM_N:(q + 1) * MM_N])
                else:
                    out_eng.dma_start(yt[c * 128:(c + 1) * 128, :], ytile[:])
    nc.compile()
    return nc


def _get_nc():
    if 'nc' not in _NC_CACHE:
        _NC_CACHE['nc'] = _build_nc()
    return _NC_CACHE['nc']


def _run(in_maps, **kwargs):
    from concourse.bass_utils import run_bass_kernel_spmd
    return run_bass_kernel_spmd(_get_nc(), in_maps, core_ids=list(range(N_CORES)),
                                **kwargs)


def _make_in_maps(state, params):
    Wtot, btot = _collapse_params(params)
    W8 = np.zeros((128, 128), np.float64)
    for f in range(8):
        W8[f * 16:(f + 1) * 16, f * 16:(f + 1) * 16] = Wtot
    W8 = W8.astype(np.float32)
    bias128 = np.tile(btot, 8).astype(np.float32).reshape(128, 1)

    state = np.asarray(state, np.float32)
    in_maps = []
    for i in range(N_CORES):
        xs = state[i * ROWS:(i + 1) * ROWS, :FLIGHT_COLS]
        in_maps.append({
            'xt': np.ascontiguousarray(xs.T),
            'w8': W8,
            'bias': bias128,
        })
    return in_maps


def _assemble(results):
    out = np.empty((B, FLIGHT_COLS), np.float32)
    for i in range(N_CORES):
        out[i * ROWS:(i + 1) * ROWS] = results[i]['yt'].T
    return out


def kernel(state, params):
    in_maps = _make_in_maps(state, params)
    last_err = None
    for attempt in range(3):
        try:
            res = _run(in_maps)
            return _assemble(res.results)
        except Exception as e:  # transient NRT device errors recover on retry
            last_err = e
            _NC_CACHE.clear()
            import time
            time.sleep(2.0 * (attempt + 1))
    raise last_err


# revision 20
# speedup vs baseline: 1.1097x; 1.0425x over previous
"""Trainium2 Bass kernel for nn_Encoder (dense_mlp).

The reference network returns only `flights_out`, which depends solely on
state[:, :1024] (64 flights x 16 features per row).  The whole flight path
(4 input encoders -> f1,f2,f3 -> df1,df2,df3 -> 4 output decoders) contains
no nonlinearity, so it collapses to a single affine map per flight vector:

    y[b, f, :] = x[b, f, :] @ Wtotal + btotal          (Wtotal: 16x16)

We evaluate it as a 128-wide matmul by block-diagonalizing 8 flights:
W8 = blockdiag(Wtotal x 8) [128, 128].  Each core handles 2048 batch rows:

    yT[128c:128c+128, rows] = W8.T @ xT[128c:128c+128, rows] + bias

Host passes xT (feature-major transposed slice) so the device only runs
full-rate streaming matmuls with W8 stationary; outputs come back
feature-major and are transposed on host.
"""

import numpy as np

B = 16384
N_CORES = 8
ROWS = B // N_CORES          # 2048 rows per core
FLIGHT_COLS = 1024           # 64 flights * 16 features
NCHUNK = FLIGHT_COLS // 128  # 8 feature chunks of 128
MM_N = 512                   # fp32 moving-operand max per matmul
NSPLIT = ROWS // MM_N        # 4 matmuls per chunk

_NC_CACHE = {}


def _collapse_params(params):
    """Fold the whole affine chain into (Wtotal [16,16], btotal [16]) in f64."""
    def g(name):
        W, b = params[name]
        return np.asarray(W, np.float64), np.asarray(b, np.float64)

    Wenc = np.zeros((16, 16)); benc = np.zeros(16)
    Wa, ba = g('airConv'); Wf, bf = g('flightTypeConv')
    Wt, bt = g('timeConv'); We, be = g('etaConv')
    Wenc[0:4, 0:4] = Wa;   benc[0:4] = ba
    Wenc[4:14, 4:8] = Wf;  benc[4:8] = bf
    Wenc[14:15, 8:12] = Wt; benc[8:12] = bt
    Wenc[15:16, 12:16] = We; benc[12:16] = be

    W, b = Wenc, benc
    for name in ('f1', 'f2', 'f3', 'df1', 'df2', 'df3'):
        Wn, bn = g(name)
        W = W @ Wn
        b = b @ Wn + bn

    Wdec = np.zeros((16, 16)); bdec = np.zeros(16)
    Wa, ba = g('airDe'); Wf, bf = g('ftDe'); Wt, bt = g('timeDe'); We, be = g('etaDe')
    Wdec[0:4, 0:4] = Wa;    bdec[0:4] = ba
    Wdec[4:8, 4:14] = Wf;   bdec[4:14] = bf
    Wdec[8:12, 14:15] = Wt; bdec[14:15] = bt
    Wdec[12:16, 15:16] = We; bdec[15:16] = be

    W = W @ Wdec
    b = b @ Wdec + bdec
    return W, b


def _build_nc(xin_bufs=6, yout_bufs=6, mm_dtype='float32', copy_split=False, out_engine='scalar',
              in_split_chunks=0, out_split=False):
    import concourse.bacc as bacc
    import concourse.tile as tile
    import concourse.mybir as mybir

    nc = bacc.Bacc(trn_type="TRN2")
    f32 = mybir.dt.float32
    # float32r is bit-identical fp32 in memory but streams through the PE at
    # 1 cycle/row (vs 4 for float32) when the moving dim is >= 256
    mdt = getattr(mybir.dt, mm_dtype)
    xt = nc.dram_tensor("xt", [FLIGHT_COLS, ROWS], mdt, kind="ExternalInput")
    w8 = nc.dram_tensor("w8", [128, 128], mdt, kind="ExternalInput")
    bias = nc.dram_tensor("bias", [128, 1], f32, kind="ExternalInput")
    yt = nc.dram_tensor("yt", [FLIGHT_COLS, ROWS], f32, kind="ExternalOutput")

    with tile.TileContext(nc) as tc:
        with tc.tile_pool(name="const", bufs=1) as cpool, \
             tc.tile_pool(name="xin", bufs=xin_bufs) as xpool, \
             tc.tile_pool(name="yout", bufs=yout_bufs) as ypool, \
             tc.tile_pool(name="ps", bufs=8, space="PSUM") as pspool:
            wt = cpool.tile([128, 128], mdt)
            nc.sync.dma_start(wt[:], w8[:, :])
            bt = cpool.tile([128, 1], f32)
            nc.sync.dma_start(bt[:], bias[:, :])
            for c in range(NCHUNK):
                # each [128, ROWS] chunk of xt/yt is one fully-contiguous
                # 1 MiB DRAM block -> peak DMA efficiency
                xtile = xpool.tile([128, ROWS], mdt)
                if c < in_split_chunks:
                    # quarter the leading chunks across 4 DMA queues so they
                    # land in chunk order at full aggregate rate (a single
                    # 1 MiB transfer rides one queue at ~100-130 GB/s and
                    # delays the first matmul by ~4 us)
                    for q in range(4):
                        nc.sync.dma_start(
                            xtile[:, q * MM_N:(q + 1) * MM_N],
                            xt[c * 128:(c + 1) * 128, q * MM_N:(q + 1) * MM_N])
                else:
                    nc.sync.dma_start(xtile[:], xt[c * 128:(c + 1) * 128, :])
                ytile = ypool.tile([128, ROWS], f32)
                for n in range(NSPLIT):
                    ps = pspool.tile([128, MM_N], f32)
                    nc.tensor.matmul(
                        ps[:], wt[:], xtile[:, n * MM_N:(n + 1) * MM_N],
                        start=True, stop=True,
                    )
                    dst = ytile[:, n * MM_N:(n + 1) * MM_N]
                    # split the PSUM->SBUF bias-copies across DVE and ACT so
                    # neither engine paces out-tile production
                    if (not copy_split) or n % 2 == 0:
                        nc.vector.tensor_scalar_add(dst, ps[:], bt[:])
                    else:
                        nc.scalar.activation(
                            dst, ps[:],
                            mybir.ActivationFunctionType.Identity, bias=bt[:])
                # out-DMAs issue via GpSimd's SWDGE queue: keeps their
                # dependency waits off the Sync (in-DMA) and Scalar (copy)
                # queues entirely
                out_eng = getattr(nc, out_engine)
                if out_split:
                    # quartered outs spread across queues: smoother out
                    # stream and a 4x-parallel final drain
                    for q in range(4):
                        out_eng.dma_start(
                            yt[c * 128:(c + 1) * 128, q * MM_N:(q + 1) * MM_N],
                            ytile[:, q * MM_N:(q + 1) * MM_N])
                else:
                    out_eng.dma_start(yt[c * 128:(c + 1) * 128, :], ytile[:])
    nc.compile()
    return nc


def _get_nc():
    if 'nc' not in _NC_CACHE:
        _NC_CACHE['nc'] = _build_nc()
    return _NC_CACHE['nc']


def _run(in_maps, **kwargs):
    from concourse.bass_utils import run_bass_kernel_spmd
    return run_bass_kernel_spmd(_get_nc(), in_maps, core_ids=list(range(N_CORES)),
                                **kwargs)


def _make_in_maps(state, params):
    Wtot, btot = _collapse_params(params)
    W8 = np.zeros((128, 128), np.float64)
    for f in range(8):
        W8[f * 16:(f + 1) * 16, f * 16:(f + 1) * 16] = Wtot
    W8 = W8.astype(np.float32)
    bias128 = np.tile(btot, 8).astype(np.float32).reshape(128, 1)

    state = np.asarray(state, np.float32)
    in_maps = []
    for i in range(N_CORES):
        xs = state[i * ROWS:(i + 1) * ROWS, :FLIGHT_COLS]
        in_maps.append({
            'xt': np.ascontiguousarray(xs.T),
            'w8': W8,
            'bias': bias128,
        })
    return in_maps


def _assemble(results):
    out = np.empty((B, FLIGHT_COLS), np.float32)
    for i in range(N_CORES):
        out[i * ROWS:(i + 1) * ROWS] = results[i]['yt'].T
    return out


def kernel(state, params):
    in_maps = _make_in_maps(state, params)
    last_err = None
    for attempt in range(3):
        try:
            res = _run(in_maps)
            return _assemble(res.results)
        except Exception as e:  # transient NRT device errors recover on retry
            last_err = e
            _NC_CACHE.clear()
            import time
            time.sleep(2.0 * (attempt + 1))
    raise last_err
